# revision 1
# baseline (speedup 1.0000x reference)
"""Trainium2 Bass kernel for the cost-volume problem.

Math: for each disparity offset r_d (9 values, spacing 0.1), warp feat_ls by
+disp and feat_rs by -disp with 1-D linear interpolation along W (the y
coordinate is always integral in the reference, so bilinear degenerates to a
1-D lerp).  With t = disp + r_d and tri(u) = max(0, 1-|u|):

    wl[w] = sum_k tri(t-k) * L[w+k]      (zero-padded L)
    wr[w] = sum_k tri(t-k) * R[w-k]      (same weights, mirrored taps)

Since disp_init is uniform in [0,1):  t in (-0.4, 1.4), and
  d<4  (r<0):  taps k in {-1,0,1}
  d==4 (r=0):  taps k in {0,1}
  d>4  (r>0):  taps k in {0,1,2}

Cost: var = ((a-b)^2 + (b-f)^2 + (f-a)^2)/9 with a=wl, b=wr, f=feat_ref.
With d1=a-b, d2=b-f:  var = 2*(d1^2 + d2^2 + d1*d2)/9, and the group mean
over 4 channels gives  out = (1/18) * sum_{c in group} (d1^2 + d2^2 + d1*d2).

Sharding: 8 cores = (B=4) x (H halves of 128 rows).  Warping is along W only,
so H-sharding needs no halo and no collectives.  Layout on-chip: partitions =
128 H rows, free dim = (channel, W).

Engine split (tuned via cost-model timeline sim): warps + diffs + cross on
VectorE (bf16 tensor_tensor, 2x mode), squares on ScalarE, accumulation tail
(ss-adds + group tree-reduce) on GpSimd. Bacc's generate_event_semaphores
legalizes multi-wait joins.
"""

import math

import numpy as np

RES = [-0.4, -0.3, -0.2, -0.1, 0.0, 0.1, 0.2, 0.3, 0.4]
C, H, W, D, G = 32, 128, 256, 9, 8
CB = 8            # channels per block
NCB = C // CB     # 4 blocks
WP = 264          # padded per-channel width (data at col offset 2)
N_CORES = 8
import os
POOL_DS = tuple(int(x) for x in os.environ.get("KPOOL_DS", "99").split(","))
TAIL_ON_POOL = os.environ.get("KTAIL_POOL", "1") == "1"

SQ_SCALE = math.sqrt(1.0 / 18.0)
CROSS_SCALE = 1.0 / 18.0

_CACHE = {}


def _taps(d):
    if d < 4:
        return (-1, 0, 1)
    if d == 4:
        return (0, 1)
    return (0, 1, 2)


def _build():
    import concourse.bacc as bacc
    import concourse.mybir as mybir
    from concourse.tile import TileContext

    f32 = mybir.dt.float32
    bf16 = mybir.dt.bfloat16
    Alu = mybir.AluOpType
    Act = mybir.ActivationFunctionType

    nc = bacc.Bacc()

    fls = nc.dram_tensor("feat_ls", [C, H, W], bf16, kind="ExternalInput")
    frs = nc.dram_tensor("feat_rs", [C, H, W], bf16, kind="ExternalInput")
    frf = nc.dram_tensor("feat_ref", [C, H, W], bf16, kind="ExternalInput")
    dsp = nc.dram_tensor("disp", [H, W], f32, kind="ExternalInput")
    out = nc.dram_tensor("out", [G, D, H, W], f32, kind="ExternalOutput")

    with TileContext(nc) as tc:
        with (
            tc.tile_pool(name="pers", bufs=1) as pers,
            tc.tile_pool(name="wts", bufs=2) as wts,
            tc.tile_pool(name="tmp", bufs=2) as tmp,
            tc.tile_pool(name="outp", bufs=3) as outp,
        ):
            # ---- per-partition bias constants for activation()
            bias_tiles = {}

            def bias_ap(v):
                v = round(float(v), 6)
                if v == 0.0:
                    return 0.0  # pre-registered const
                if v not in bias_tiles:
                    bt = pers.tile([H, 1], f32, name=f"bias_{len(bias_tiles)}",
                                   tag=f"bias{len(bias_tiles)}")
                    nc.vector.memset(bt[:], v)
                    bias_tiles[v] = bt
                return bias_tiles[v][:]

            # ---- disp: load and replicate to [H, CB, W] (f32)
            dtile = pers.tile([H, W], f32)
            nc.sync.dma_start(out=dtile[:], in_=dsp[:])
            drep = pers.tile([H, CB, W], f32)
            nc.vector.tensor_copy(drep[:, 0, :], dtile[:])
            nc.vector.tensor_copy(drep[:, 1, :], dtile[:])
            nc.vector.tensor_copy(drep[:, 2:4, :], drep[:, 0:2, :])
            nc.vector.tensor_copy(drep[:, 4:8, :], drep[:, 0:4, :])

            # ---- features (already bf16 in DRAM), padded at two phases
            Lp2 = pers.tile([H, C, WP], bf16)
            Lp3 = pers.tile([H, C, WP], bf16)
            Rp2 = pers.tile([H, C, WP], bf16)
            Rp3 = pers.tile([H, C, WP], bf16)
            Fb = pers.tile([H, C, W], bf16)
            for t in (Lp2, Lp3, Rp2, Rp3):
                nc.vector.memset(t[:], 0.0)  # pads stay 0
            for src, p2, p3 in ((fls, Lp2, Lp3), (frs, Rp2, Rp3)):
                hcw = src[:].rearrange("c h w -> h c w")
                nc.sync.dma_start(out=p2[:, :, 2:258], in_=hcw)
                nc.sync.dma_start(out=p3[:, :, 3:259], in_=hcw)
            nc.sync.dma_start(out=Fb[:], in_=frf[:].rearrange("c h w -> h c w"))

            def lview(k, c0):
                src, off = (Lp2, 2 + k) if k % 2 == 0 else (Lp3, 3 + k)
                return src[:, c0:c0 + CB, off:off + W]

            def rview(k, c0):
                src, off = (Rp2, 2 - k) if k % 2 == 0 else (Rp3, 3 - k)
                return src[:, c0:c0 + CB, off:off + W]

            # ---- main loop over disparities
            for d in range(D):
                r = float(RES[d])
                taps = _taps(d)
                wk = {}
                if d < 4:
                    wm1 = wts.tile([H, CB, W], bf16, tag="wA")
                    nc.scalar.activation(wm1[:], drep[:], Act.Relu,
                                         bias=bias_ap(-r), scale=-1.0)
                    wu = wts.tile([H, CB, W], bf16, tag="wU", bufs=1)
                    nc.scalar.activation(wu[:], drep[:], Act.Abs,
                                         bias=bias_ap(r), scale=1.0)
                    w0 = wts.tile([H, CB, W], bf16, tag="wB")
                    nc.scalar.activation(w0[:], wu[:], Act.Relu,
                                         bias=bias_ap(1.0), scale=-1.0)
                    w1 = wts.tile([H, CB, W], bf16, tag="wC")
                    nc.scalar.activation(w1[:], drep[:], Act.Relu,
                                         bias=bias_ap(r), scale=1.0)
                    wk = {-1: wm1, 0: w0, 1: w1}
                elif d == 4:
                    w0 = wts.tile([H, CB, W], bf16, tag="wB")
                    nc.scalar.activation(w0[:], drep[:], Act.Relu,
                                         bias=bias_ap(1.0), scale=-1.0)
                    w1 = wts.tile([H, CB, W], bf16, tag="wC")
                    nc.scalar.activation(w1[:], drep[:], Act.Relu,
                                         bias=0.0, scale=1.0)
                    wk = {0: w0, 1: w1}
                else:
                    w0 = wts.tile([H, CB, W], bf16, tag="wB")
                    nc.scalar.activation(w0[:], drep[:], Act.Relu,
                                         bias=bias_ap(1.0 - r), scale=-1.0)
                    wu = wts.tile([H, CB, W], bf16, tag="wU", bufs=1)
                    nc.scalar.activation(wu[:], drep[:], Act.Abs,
                                         bias=bias_ap(r - 1.0), scale=1.0)
                    w1 = wts.tile([H, CB, W], bf16, tag="wC")
                    nc.scalar.activation(w1[:], wu[:], Act.Relu,
                                         bias=bias_ap(1.0), scale=-1.0)
                    w2 = wts.tile([H, CB, W], bf16, tag="wA")
                    nc.scalar.activation(w2[:], drep[:], Act.Relu,
                                         bias=bias_ap(r - 1.0), scale=1.0)
                    wk = {0: w0, 1: w1, 2: w2}

                eng = nc.gpsimd if d in POOL_DS else nc.vector
                sfx = "g" if d in POOL_DS else "v"

                nb = int(os.environ.get("KNB", "4")) if sfx == "v" else 1
                for cb in range(NCB):
                    c0 = cb * CB

                    wl = tmp.tile([H, CB, W], bf16, tag=f"wl{sfx}", bufs=nb)
                    wr = tmp.tile([H, CB, W], bf16, tag=f"wr{sfx}", bufs=nb)
                    mm = tmp.tile([H, CB, W], bf16, tag=f"mm{sfx}", bufs=nb)

                    k0, k1 = taps[0], taps[1]
                    eng.tensor_mul(wl[:], wk[k0][:], lview(k0, c0))
                    eng.tensor_mul(mm[:], wk[k1][:], lview(k1, c0))
                    eng.tensor_add(wl[:], wl[:], mm[:])
                    eng.tensor_mul(wr[:], wk[k0][:], rview(k0, c0))
                    eng.tensor_mul(mm[:], wk[k1][:], rview(k1, c0))
                    eng.tensor_add(wr[:], wr[:], mm[:])
                    if len(taps) == 3:
                        k2 = taps[2]
                        eng.tensor_mul(mm[:], wk[k2][:], lview(k2, c0))
                        eng.tensor_add(wl[:], wl[:], mm[:])
                        eng.tensor_mul(mm[:], wk[k2][:], rview(k2, c0))
                        eng.tensor_add(wr[:], wr[:], mm[:])

                    # in place: wl <- d1 = wl - wr ; wr <- d2 = wr - F
                    se = nc.gpsimd if os.environ.get("KSUB_POOL", "0") == "1"                         else eng
                    se.tensor_sub(wl[:], wl[:], wr[:])
                    se.tensor_sub(wr[:], wr[:], Fb[:, c0:c0 + CB, :])

                    sq1 = tmp.tile([H, CB, W], bf16, tag=f"sq1{sfx}", bufs=int(os.environ.get("KSQB", "2")))
                    sq2 = tmp.tile([H, CB, W], bf16, tag=f"sq2{sfx}", bufs=int(os.environ.get("KSQB", "2")))
                    if sfx == "v":
                        nc.scalar.activation(sq1[:], wl[:], Act.Square,
                                             scale=SQ_SCALE)
                        nc.scalar.activation(sq2[:], wr[:], Act.Square,
                                             scale=SQ_SCALE)
                        # mm <- cross = (d1 * 1/18) * d2
                        eng.scalar_tensor_tensor(
                            mm[:], wl[:], CROSS_SCALE, wr[:],
                            Alu.mult, Alu.mult)
                    else:
                        # Pool cannot run TensorScalarPtr: use d3 form,
                        # out = (d1^2 + d2^2 + d3^2)/36 with d3 = d1 + d2
                        sc = math.sqrt(1.0 / 36.0)
                        nc.scalar.activation(sq1[:], wl[:], Act.Square,
                                             scale=sc)
                        nc.scalar.activation(sq2[:], wr[:], Act.Square,
                                             scale=sc)
                        eng.tensor_add(mm[:], wl[:], wr[:])  # d3
                        nc.scalar.activation(mm[:], mm[:], Act.Square,
                                             scale=sc)
                    te = nc.gpsimd if (TAIL_ON_POOL and sfx == "v") else eng
                    # sq1 <- ss = sq1 + sq2 + cross|sq3
                    te.tensor_add(sq1[:], sq1[:], sq2[:])
                    te.tensor_add(sq1[:], sq1[:], mm[:])

                    sv = sq1[:].rearrange("p (g c) w -> p g c w", c=4)
                    t1 = tmp.tile([H, 2, 2, W], bf16, tag=f"t1{sfx}", bufs=nb)
                    te.tensor_add(t1[:], sv[:, :, 0:2, :], sv[:, :, 2:4, :])
                    res = outp.tile([H, 2, W], f32, tag=f"res{sfx}",
                                    bufs=3 if sfx == "v" else 2)
                    te.tensor_add(res[:], t1[:, :, 0, :], t1[:, :, 1, :])
                    nc.sync.dma_start(
                        out=out[cb * 2:cb * 2 + 2, d].rearrange(
                            "g h w -> h g w"),
                        in_=res[:])
    nc.finalize()
    return nc


def _get_nc():
    if "nc" not in _CACHE:
        _CACHE["nc"] = _build()
    return _CACHE["nc"]


def make_in_maps(feat_ref, feat_ls, feat_rs, disp_init):
    import ml_dtypes
    bf = ml_dtypes.bfloat16
    in_maps = []
    for core in range(N_CORES):
        b, hh = core // 2, core % 2
        h0 = hh * H
        in_maps.append({
            "feat_ls": np.ascontiguousarray(
                feat_ls[b, :, h0:h0 + H, :]).astype(bf),
            "feat_rs": np.ascontiguousarray(
                feat_rs[b, :, h0:h0 + H, :]).astype(bf),
            "feat_ref": np.ascontiguousarray(
                feat_ref[b, :, h0:h0 + H, :]).astype(bf),
            "disp": np.ascontiguousarray(
                disp_init[b, 0, h0:h0 + H, :], dtype=np.float32),
        })
    return in_maps


def assemble(results):
    full = np.zeros((4, G, D, 2 * H, W), np.float32)
    for core in range(N_CORES):
        b, hh = core // 2, core % 2
        full[b, :, :, hh * H:(hh + 1) * H, :] = results[core]["out"]
    return full


def run(feat_ref, feat_ls, feat_rs, disp_init, trace=False):
    from concourse.bass_utils import run_bass_kernel_spmd

    in_maps = make_in_maps(feat_ref, feat_ls, feat_rs, disp_init)
    r = run_bass_kernel_spmd(
        _get_nc(), in_maps, core_ids=list(range(N_CORES)), trace=trace)
    return assemble(r.results), r


def kernel(feat_ref, feat_ls, feat_rs, disp_init):
    out, _ = run(feat_ref, feat_ls, feat_rs, disp_init)
    return out



# revision 10
# speedup vs baseline: 2.4726x; 2.4726x over previous
"""Trainium2 Bass kernel for the cost-volume problem (coefficient scheme).

Math: y-coords are integral so the bilerp degenerates to a 1-D lerp along W.
With t = disp + r_d, tri-weights summing to 1 over the tap set, and
zero-padded L/R:

    a_d = wl, b_d = wr,  var-sum = (1/18) sum_c [ (a-b)^2+(b-f)^2+(f-a)^2 ]/2
    d1^2+d2^2+d1*d2 = 1/4 (a+b-2f)^2 + 3/4 (a-b)^2

Define (host-prescaled so the 1/18 and 1/4, 3/4 constants vanish):
    U_k = Lh[+k] + Rh[-k] - Fs      (Lh = 0.5*s18*L, Fs = s18*F)
    V_k = Ls[+k] - Rs[-k]           (Ls = (sqrt3/2)*s18*L),  s18 = 1/sqrt(18)
    out[g,d] = sum_{c in g} u(t)^2 + v(t)^2,  u = sum_k w_k(t) U_k  etc.

u(t) is piecewise linear in t with kinks at t=0,1, so q(t) = u^2+v^2 summed
over the group is an exact piecewise QUADRATIC in t with 3 pieces.  Per group
pixel we build 7 coefficient maps:

    A = S(U0^2+V0^2), B = 2*SB, C = SC           (mid piece, t in [0,1])
    SB = S(U0*DU+ + V0*DV+), SC = S(DU+^2+DV+^2)
    corr left  (t<0): + 2r*lamL + r^2*muL,  r = relu(-t)
         lamL = SPm + SB,       muL = SQm - SC
    corr right (t>1): + 2s*lamR + s^2*muR,  s = relu(t-1)
         lamR = SP1 - SB - SC,  muR = SQ1 - SC
    (DU+ = U1-U0, DU- = U-1 - U0, DU1 = U2-U1; SPm,SQm,SP1,SQ1 analogous)

Since t_d = t_0 + 0.1d, the mid-piece q(t_d) follows a 2nd-difference
recurrence: acc += delta; delta += 0.02C  -- 2 cheap TTs per disparity
instead of a full warp+variance pass.  d=4 has no corrections: its output
is DMA'd straight from the accumulator.

Sharding: 8 cores = (B=4) x (H halves of 128 rows); warping is along W so
H-sharding needs no halo.  Layout: partitions = 128 H rows.

Engines: blocks/diffs/crosses/tree-L1/eval on DVE (bf16 2x mode, weight maps
broadcast via stride-0 APs), squares + weight maps on Act, tree-L2/L3 +
memsets on Pool.
"""

import math
import os

import numpy as np

RES = [-0.4, -0.3, -0.2, -0.1, 0.0, 0.1, 0.2, 0.3, 0.4]
C, H, W, D, G = 32, 128, 256, 9, 8
CB = 8            # channels per block
NCB = C // CB     # 4 blocks
WP = 264          # padded per-channel width (data at col offset 2)
N_CORES = 8

S18 = 1.0 / math.sqrt(18.0)
SC_U = 0.5 * S18            # scale for Lh, Rh
SC_V = math.sqrt(3.0) / 2.0 * S18   # scale for Ls, Rs

_CACHE = {}

# engine knobs (tuned via timeline sim)
ENG_L1 = os.environ.get("KL1", "dve")
ENG_L23 = os.environ.get("KL23", "pool")
ENG_CROSS = os.environ.get("KCROSS", "dve")
ENG_CORR = os.environ.get("KCORR", "dve")


def _build():
    import concourse.bacc as bacc
    import concourse.mybir as mybir
    from concourse.bass import AP
    from concourse.tile import TileContext

    f32 = mybir.dt.float32
    bf16 = mybir.dt.bfloat16
    Act = mybir.ActivationFunctionType

    nc = bacc.Bacc()

    lh = nc.dram_tensor("lh", [C, H, W], bf16, kind="ExternalInput")
    rh = nc.dram_tensor("rh", [C, H, W], bf16, kind="ExternalInput")
    lsg = nc.dram_tensor("lsg", [C, H, W], bf16, kind="ExternalInput")
    rsg = nc.dram_tensor("rsg", [C, H, W], bf16, kind="ExternalInput")
    fsd = nc.dram_tensor("fs", [C, H, W], bf16, kind="ExternalInput")
    dsp = nc.dram_tensor("disp", [H, W], f32, kind="ExternalInput")
    out = nc.dram_tensor("out", [G, D, H, W], bf16, kind="ExternalOutput")

    def eng(name):
        return {"dve": nc.vector, "pool": nc.gpsimd, "act": nc.scalar}[name]

    with TileContext(nc) as tc:
        with (
            tc.tile_pool(name="pers", bufs=1) as pers,
            tc.tile_pool(name="inp", bufs=2) as inp,
            tc.tile_pool(name="blk", bufs=1) as blk,
            tc.tile_pool(name="prod", bufs=2) as prod,
            tc.tile_pool(name="tree", bufs=2) as tree,
            tc.tile_pool(name="coef", bufs=2) as coefp,
            tc.tile_pool(name="outp", bufs=4) as outp,
        ):
            # ---------- disp + compact weight maps [H,1,W] ----------
            dt_ = pers.tile([H, 1, W], f32)
            nc.sync.dma_start(out=dt_[:, 0, :], in_=dsp[:])
            dv = dt_[:]

            bias_tiles = {}

            def bias_ap(v):
                v = round(float(v), 6)
                if v == 0.0:
                    return 0.0
                if v not in bias_tiles:
                    bt = pers.tile([H, 1], f32, name=f"bias_{len(bias_tiles)}",
                                   tag=f"bias{len(bias_tiles)}")
                    nc.vector.memset(bt[:], v)
                    bias_tiles[v] = bt
                return bias_tiles[v][:]

            def wmap(name, func, scale, bias, src=None):
                t = pers.tile([H, 1, W], bf16, name=name, tag=name)
                b = (float(bias) if func == Act.Copy else bias_ap(bias))
                nc.scalar.activation(t[:], src if src is not None else dv,
                                     func, bias=b, scale=float(scale))
                return t

            w2t0 = wmap("w2t0", Act.Copy, 2.0, -0.8)
            wt0sq = wmap("wt0sq", Act.Square, 1.0, -0.4)
            wdel = wmap("wdel", Act.Copy, 0.2, -0.07)
            w1_, w2_ = {}, {}
            for d in range(4):          # left corr: r = relu(-t)
                w1_[d] = wmap(f"w2r{d}", Act.Relu, -2.0, -2.0 * RES[d])
                w2_[d] = wmap(f"wr2{d}", Act.Square, 0.5, 0.0, src=w1_[d][:])
            for d in range(5, 9):       # right corr: s = relu(t-1)
                w1_[d] = wmap(f"w2s{d}", Act.Relu, 2.0, 2.0 * (RES[d] - 1.0))
                w2_[d] = wmap(f"ws2{d}", Act.Square, 0.5, 0.0, src=w1_[d][:])

            def bc(wt, n):
                """broadcast [H,1,W] weight tile over a middle dim of size n"""
                a = wt[:]
                return AP(a.tensor, a.offset,
                          [list(a.ap[0]), [0, n], list(a.ap[2])])

            for cb in range(NCB):
                c0 = cb * CB
                g0 = cb * 2      # two groups per channel block

                # ---- padded inputs [H, CB, WP], data at cols 2..257
                pads = {}
                for nm, src in (("lhp", lh), ("rhp", rh),
                                ("lsp", lsg), ("rsp", rsg)):
                    t = inp.tile([H, CB, WP], bf16, tag=nm)
                    nc.gpsimd.memset(t[:, :, 0:2], 0.0)
                    nc.gpsimd.memset(t[:, :, 258:264], 0.0)
                    nc.sync.dma_start(
                        out=t[:, :, 2:258],
                        in_=src[c0:c0 + CB].rearrange("c h w -> h c w"))
                    pads[nm] = t
                fcb = inp.tile([H, CB, W], bf16, tag="fcb")
                nc.sync.dma_start(
                    out=fcb[:],
                    in_=fsd[c0:c0 + CB].rearrange("c h w -> h c w"))

                def shifted(tile, base_col, kstride):
                    """[H, 4k, CB, W] overlapping view of a padded tile"""
                    a = tile[:]
                    p = list(a.ap[0])
                    return AP(a.tensor, a.offset + base_col,
                              [p, [kstride, 4], [WP, CB], [1, W]])

                # ---- blocks: Ublk/Vblk [H, 4k, CB, W], k = -1,0,1,2
                ublk = blk.tile([H, 4, CB, W], bf16, tag="ublk")
                vblk = blk.tile([H, 4, CB, W], bf16, tag="vblk")
                fa = fcb[:]
                fbc = AP(fa.tensor, fa.offset,
                         [list(fa.ap[0]), [0, 4], list(fa.ap[1]),
                          list(fa.ap[2])])
                nc.vector.tensor_add(ublk[:], shifted(pads["lhp"], 1, 1),
                                     shifted(pads["rhp"], 3, -1))
                nc.vector.tensor_sub(ublk[:], ublk[:], fbc)
                nc.vector.tensor_sub(vblk[:], shifted(pads["lsp"], 1, 1),
                                     shifted(pads["rsp"], 3, -1))

                # ---- diffs: Ddual [H, 2, CB, W] = (D+ , D-), D1 [H, CB, W]
                def kview(t, idx, n, step):
                    a = t[:]
                    kst = a.ap[1][0]
                    return AP(a.tensor, a.offset + idx * kst,
                              [list(a.ap[0]), [step * kst, n],
                               list(a.ap[2]), list(a.ap[3])])

                dd, d1 = {}, {}
                for nm, bt in (("u", ublk), ("v", vblk)):
                    ddt = blk.tile([H, 2, CB, W], bf16, tag=f"dd{nm}")
                    # in0: k indices [2, 0]; in1: k=1 broadcast
                    nc.vector.tensor_sub(ddt[:], kview(bt, 2, 2, -2),
                                         kview(bt, 1, 2, 0))
                    d1t = blk.tile([H, CB, W], bf16, tag=f"d1{nm}")
                    nc.vector.tensor_sub(
                        d1t[:], bt[:, 3], bt[:, 2])
                    dd[nm], d1[nm] = ddt, d1t

                u0 = ublk[:, 1].rearrange("h (g c) w -> h g c w", g=2)
                u1 = ublk[:, 2].rearrange("h (g c) w -> h g c w", g=2)
                v0 = vblk[:, 1].rearrange("h (g c) w -> h g c w", g=2)
                v1 = vblk[:, 2].rearrange("h (g c) w -> h g c w", g=2)
                dup = dd["u"][:, 0].rearrange("h (g c) w -> h g c w", g=2)
                dum = dd["u"][:, 1].rearrange("h (g c) w -> h g c w", g=2)
                dvp = dd["v"][:, 0].rearrange("h (g c) w -> h g c w", g=2)
                dvm = dd["v"][:, 1].rearrange("h (g c) w -> h g c w", g=2)
                d1u = d1["u"][:].rearrange("h (g c) w -> h g c w", g=2)
                d1v = d1["v"][:].rearrange("h (g c) w -> h g c w", g=2)

                # ---- products + tree-reduce into S maps [H, 2, W]
                ce = eng(ENG_CROSS)

                S = {}
                for nm in ("sa", "sb", "sc", "spm", "sqm", "sp1", "sq1"):
                    S[nm] = coefp.tile([H, 2, W], bf16, name=nm, tag=nm)

                def coeff(mk_u, mk_v, dst):
                    pj = prod.tile([H, 2, 4, 2, W], bf16, tag="pj")
                    mk_u(pj[:, :, :, 0, :])
                    mk_v(pj[:, :, :, 1, :])
                    t1 = tree.tile([H, 2, 2, 2, W], bf16, tag="t1")
                    eng(ENG_L1).tensor_add(t1[:], pj[:, :, 0:2, :, :],
                                           pj[:, :, 2:4, :, :])
                    t2 = tree.tile([H, 2, 2, W], bf16, tag="t2")
                    eng(ENG_L23).tensor_add(t2[:], t1[:, :, 0], t1[:, :, 1])
                    eng(ENG_L23).tensor_add(dst[:], t2[:, :, 0, :],
                                            t2[:, :, 1, :])

                def sq(dst_v, src_v):
                    nc.scalar.activation(dst_v, src_v, Act.Square)

                coeff(lambda o: sq(o, u0), lambda o: sq(o, v0), S["sa"])
                coeff(lambda o: ce.tensor_mul(o, u0, dup),
                      lambda o: ce.tensor_mul(o, v0, dvp), S["sb"])
                coeff(lambda o: sq(o, dup), lambda o: sq(o, dvp), S["sc"])
                coeff(lambda o: ce.tensor_mul(o, u0, dum),
                      lambda o: ce.tensor_mul(o, v0, dvm), S["spm"])
                coeff(lambda o: sq(o, dum), lambda o: sq(o, dvm), S["sqm"])
                coeff(lambda o: ce.tensor_mul(o, u1, d1u),
                      lambda o: ce.tensor_mul(o, v1, d1v), S["sp1"])
                coeff(lambda o: sq(o, d1u), lambda o: sq(o, d1v), S["sq1"])

                # ---- assembly on [H, 2, W]
                sa, sb, sc = S["sa"][:], S["sb"][:], S["sc"][:]
                spm, sqm = S["spm"][:], S["sqm"][:]
                sp1, sq1 = S["sp1"][:], S["sq1"][:]

                lamL = coefp.tile([H, 2, W], bf16, tag="lamL")
                muL = coefp.tile([H, 2, W], bf16, tag="muL")
                lamR = coefp.tile([H, 2, W], bf16, tag="lamR")
                muR = coefp.tile([H, 2, W], bf16, tag="muR")
                acc = coefp.tile([H, 2, W], bf16, tag="acc")
                dlt = coefp.tile([H, 2, W], bf16, tag="dlt")
                c2t = coefp.tile([H, 2, W], bf16, tag="c2t")

                nc.vector.tensor_add(lamL[:], spm, sb)
                nc.vector.tensor_sub(muL[:], sqm, sc)
                t_lr = tree.tile([H, 2, W], bf16, tag="tlr")
                nc.vector.tensor_sub(t_lr[:], sp1, sb)
                nc.vector.tensor_sub(lamR[:], t_lr[:], sc)
                nc.vector.tensor_sub(muR[:], sq1, sc)

                m1 = tree.tile([H, 2, W], bf16, tag="am1")
                m2 = tree.tile([H, 2, W], bf16, tag="am2")
                nc.vector.tensor_mul(m1[:], bc(w2t0, 2), sb)
                nc.vector.tensor_mul(m2[:], bc(wt0sq, 2), sc)
                nc.vector.tensor_add(acc[:], sa, m1[:])
                nc.vector.tensor_add(acc[:], acc[:], m2[:])

                # delta = 0.2*SB + wdel*SC ; c2 = 0.02*SC   (scales on Act)
                m3 = tree.tile([H, 2, W], bf16, tag="am3")
                nc.scalar.activation(m3[:], sb, Act.Copy, bias=0.0, scale=0.2)
                m4 = tree.tile([H, 2, W], bf16, tag="am4")
                nc.vector.tensor_mul(m4[:], bc(wdel, 2), sc)
                nc.vector.tensor_add(dlt[:], m3[:], m4[:])
                nc.scalar.activation(c2t[:], sc, Act.Copy,
                                     bias=0.0, scale=0.02)

                # ---- eval d = 0..8 for this channel block's two groups
                ee = eng(ENG_CORR)
                for d in range(D):
                    if d > 0:
                        nc.vector.tensor_add(acc[:], acc[:], dlt[:])
                        if d < D - 1:
                            nc.vector.tensor_add(dlt[:], dlt[:], c2t[:])
                    od = out[g0:g0 + 2, d].rearrange("g h w -> h g w")
                    if d == 4:
                        nc.sync.dma_start(out=od, in_=acc[:])
                        continue
                    la, mu = (lamL, muL) if d < 4 else (lamR, muR)
                    e1 = outp.tile([H, 2, W], bf16, tag="e1", bufs=3)
                    e2 = outp.tile([H, 2, W], bf16, tag="e2", bufs=3)
                    ee.tensor_mul(e1[:], bc(w1_[d], 2), la[:])
                    ee.tensor_mul(e2[:], bc(w2_[d], 2), mu[:])
                    ee.tensor_add(e1[:], e1[:], e2[:])
                    o = outp.tile([H, 2, W], bf16, tag="o", bufs=4)
                    nc.vector.tensor_add(o[:], acc[:], e1[:])
                    nc.sync.dma_start(out=od, in_=o[:])
    nc.finalize()
    return nc


def _get_nc():
    if "nc" not in _CACHE:
        _CACHE["nc"] = _build()
    return _CACHE["nc"]


def make_in_maps(feat_ref, feat_ls, feat_rs, disp_init):
    import ml_dtypes
    bf = ml_dtypes.bfloat16
    f32 = np.float32
    in_maps = []
    for core in range(N_CORES):
        b, hh = core // 2, core % 2
        h0 = hh * H
        L = feat_ls[b, :, h0:h0 + H, :]
        R = feat_rs[b, :, h0:h0 + H, :]
        F = feat_ref[b, :, h0:h0 + H, :]
        in_maps.append({
            "lh": np.ascontiguousarray(L * f32(SC_U)).astype(bf),
            "rh": np.ascontiguousarray(R * f32(SC_U)).astype(bf),
            "lsg": np.ascontiguousarray(L * f32(SC_V)).astype(bf),
            "rsg": np.ascontiguousarray(R * f32(SC_V)).astype(bf),
            "fs": np.ascontiguousarray(F * f32(S18)).astype(bf),
            "disp": np.ascontiguousarray(
                disp_init[b, 0, h0:h0 + H, :], dtype=f32),
        })
    return in_maps


def assemble(results):
    full = np.zeros((4, G, D, 2 * H, W), np.float32)
    for core in range(N_CORES):
        b, hh = core // 2, core % 2
        full[b, :, :, hh * H:(hh + 1) * H, :] = \
            results[core]["out"].astype(np.float32)
    return full


def run(feat_ref, feat_ls, feat_rs, disp_init, trace=False):
    from concourse.bass_utils import run_bass_kernel_spmd

    in_maps = make_in_maps(feat_ref, feat_ls, feat_rs, disp_init)
    r = run_bass_kernel_spmd(
        _get_nc(), in_maps, core_ids=list(range(N_CORES)), trace=trace)
    return assemble(r.results), r


def kernel(feat_ref, feat_ls, feat_rs, disp_init):
    out, _ = run(feat_ref, feat_ls, feat_rs, disp_init)
    return out
